# revision 17
# baseline (speedup 1.0000x reference)
"""Causal self-attention TRN2 Bass kernel (phase-interleaved).

Problem: B=4, T=2048, C=1024, H=16 heads (HD=64), torch-Linear semantics
(y = x @ W.T + b), causal + padding mask, softmax, output projection.

Sharding: 8 cores = (batch b in 0..3) x (head-half in 0..1). Each core
handles one batch and 8 heads (512 of the 1024 channels of QKV / of the
contraction dim of the output projection). The two half-cores of a batch
produce partial output projections that the host sums (plus bp).

Design (376us baseline -> ~276us):
  - Attention q-chunks run ASCENDING; PE-bound projection and
    output-projection units are interleaved as filler between the
    ACT-bound softmax steps so the Tensor engine never idles while ACT
    grinds exp(). Fill is placed by measured engine balance: rounds 1-2
    are PE-bound (proj fill only); round 3 is ACT-bound with ~17us of
    PE slack, so ALL output-projection fill (chunks 1 and 2) parks
    there:
      proj(0) -> [attn(0)|proj(1)] -> [attn(1)|proj(2)|out(0)]
              -> [attn(2)|proj(3)] -> [attn(3)|out(1)|out(2)] -> out(3)
  - All inputs shipped bf16 (same matmul rate, half the DMA bytes);
    V and exp(S) tiles bf16 (faster LDWEIGHTS, 2x DVE mask multiply).
  - DMA issue spread across the 3 capable queues (sync/scalar/gpsimd),
    ~700ns per dma_start per queue; first x/Wq tiles split in half so
    the first matmul starts ~10us in.
  - Causal diag mask: DVE multiply with a 0/1 triangle after exp
    (no PE identity-matmuls); V bias added during the DVE psum drain.
  - Rowsum via a ones-column in the V stationary ([V|1]); normalization:
    psum row -> SBUF copy -> reciprocal_approx_fast -> partition
    broadcast -> DVE multiply (approx-recip cannot read PSUM directly).
  - Y^T kept as four per-head-group tiles so output-projection reads
    depend only on the epilogue that wrote them; the last four token
    tiles use a 6-buffer psum pool (attention pools released first) so
    their partial accumulations cover the final epilogue's latency.
  - PSUM: 2 proj/out + 2x2 S + 1x2 O banks = 8 during attention.
  - Engine budget: PE ~245us (the roofline), ACT ~158us of exp,
    DVE ~130us of drains/masks/normalize, sync ~45us of DMA issue.
"""

import ml_dtypes
import numpy as np

import concourse.mybir as mybir
import concourse.tile as tile
from concourse import bacc
from concourse.bass_utils import run_bass_kernel_spmd

F32 = mybir.dt.float32
F32R = mybir.dt.float32r
BF16 = mybir.dt.bfloat16
AF = mybir.ActivationFunctionType
ALU = mybir.AluOpType

B, T, C, H = 4, 2048, 1024, 16
HD = C // H          # 64
IC = C // 2          # 512 channels per core (8 heads)
NKT = T // 128       # 16 k-tiles
NCT = C // 128       # 8 contraction tiles for QKV
NEG = -1.0e30
SCALE = 1.0 / np.sqrt(HD)
D = 8                # S->AV pipeline lag (in k-tile steps)

_CACHE = {}


def _build():
    nc = bacc.Bacc("TRN2", target_bir_lowering=False, debug=False)

    xT_d = nc.dram_tensor("xT", [C, T], BF16, kind="ExternalInput").ap()
    WqT_d = nc.dram_tensor("WqT", [C, IC], BF16, kind="ExternalInput").ap()
    WkT_d = nc.dram_tensor("WkT", [C, IC], BF16, kind="ExternalInput").ap()
    WvT_d = nc.dram_tensor("WvT", [C, IC], BF16, kind="ExternalInput").ap()
    WpT_d = nc.dram_tensor("WpT", [IC, C], BF16, kind="ExternalInput").ap()
    bq_d = nc.dram_tensor("bqs", [128, 4], F32, kind="ExternalInput").ap()
    bk_d = nc.dram_tensor("bks", [128, 4], F32, kind="ExternalInput").ap()
    bv_d = nc.dram_tensor("bvr", [1, IC], F32, kind="ExternalInput").ap()
    pad_d = nc.dram_tensor("padb", [128, NKT], F32, kind="ExternalInput").ap()
    tri_d = nc.dram_tensor("tri01", [128, 128], BF16, kind="ExternalInput").ap()
    ones_d = nc.dram_tensor("ones8", [128, 8], BF16, kind="ExternalInput").ap()
    out_d = nc.dram_tensor("out", [T, C], F32, kind="ExternalOutput").ap()

    with tile.TileContext(nc) as tc:
        with tc.tile_pool(name="pp", bufs=1) as pp:
            QT = pp.tile([128, 4 * T], BF16, name="QT")
            KT = pp.tile([128, 4 * T], BF16, name="KT")
            Vt = pp.tile([128, NKT * 520], BF16, name="Vt")
            YTg = [pp.tile([128, T], BF16, name=f"YT{i}", uniquify=False)
                   for i in range(4)]
            Wp_sb = pp.tile([128, 4 * C], BF16, name="Wp_sb")
            Wq_sb = pp.tile([128, NCT * 512], BF16, name="Wq_sb")
            Wk_sb = pp.tile([128, NCT * 512], BF16, name="Wk_sb")
            Wv_sb = pp.tile([128, NCT * 512], BF16, name="Wv_sb")
            bq_sb = pp.tile([128, 4], F32, name="bq_sb")
            bk_sb = pp.tile([128, 4], F32, name="bk_sb")
            bv_sb = pp.tile([1, IC], F32, name="bv_sb")
            bvb_sb = pp.tile([128, IC], F32, name="bvb_sb")
            pad_sb = pp.tile([128, NKT], F32, name="pad_sb")
            tri_sb = pp.tile([128, 128], BF16, name="tri_sb")
            one8_sb = pp.tile([128, 8], BF16, name="one8_sb")

            Vf = Vt.rearrange("p (k h c) -> p k h c", k=NKT, h=8, c=65)
            nc.gpsimd.dma_start(out=one8_sb[:], in_=ones_d)
            for kt in range(NKT):
                nc.vector.tensor_copy(Vf[:, kt, :, 64], one8_sb[:, :, None])

            xs = tc.alloc_tile_pool(name="xs", bufs=2)
            ps1 = tc.alloc_tile_pool(name="ps1", bufs=2, space="PSUM")
            pss = tc.alloc_tile_pool(name="pss", bufs=2, space="PSUM")
            pso = tc.alloc_tile_pool(name="pso", bufs=1, space="PSUM")
            es = tc.alloc_tile_pool(name="es", bufs=D + 1)
            rp = tc.alloc_tile_pool(name="rp", bufs=2)
            obp = tc.alloc_tile_pool(name="ob", bufs=3)

            xc_t = [None] * 4

            def load_x(ch):
                xc = xs.tile([128, NCT * 512], BF16, name="xc", tag="xc")
                t0 = ch * 512
                for ct in range(NCT):
                    nc.sync.dma_start(
                        out=xc[:, ct * 512:(ct + 1) * 512],
                        in_=xT_d[ct * 128:(ct + 1) * 128, t0:t0 + 512])
                xc_t[ch] = xc

            # ---- head: first chunk loads spread over 3 issue queues;
            # first two column-tiles split in half so the first proj
            # matmuls can start ~3us earlier
            xc0 = xs.tile([128, NCT * 512], BF16, name="xc", tag="xc")
            xc_t[0] = xc0
            for ct in range(NCT):
                cs = slice(ct * 128, (ct + 1) * 128)
                if ct < 2:
                    for hh in range(2):
                        nc.sync.dma_start(
                            out=xc0[:, ct * 512 + hh * 256: ct * 512 + hh * 256 + 256],
                            in_=xT_d[cs, hh * 256:hh * 256 + 256])
                        nc.scalar.dma_start(
                            out=Wq_sb[:, ct * 512 + hh * 256: ct * 512 + hh * 256 + 256],
                            in_=WqT_d[ct * 128 + hh * 64: ct * 128 + (hh + 1) * 64, :].rearrange("(a b) c -> a (b c)", a=128, b=1) if False else WqT_d[cs, hh * 256:hh * 256 + 256])
                else:
                    nc.sync.dma_start(out=xc0[:, ct * 512:(ct + 1) * 512],
                                      in_=xT_d[cs, 0:512])
                    nc.scalar.dma_start(out=Wq_sb[:, ct * 512:(ct + 1) * 512],
                                        in_=WqT_d[cs, :])
                nc.gpsimd.dma_start(out=Wk_sb[:, ct * 512:(ct + 1) * 512],
                                    in_=WkT_d[cs, :])
            for ct in range(NCT):
                nc.sync.dma_start(out=Wv_sb[:, ct * 512:(ct + 1) * 512],
                                  in_=WvT_d[ct * 128:(ct + 1) * 128, :])
            nc.scalar.dma_start(out=bq_sb[:], in_=bq_d)
            nc.scalar.dma_start(out=bk_sb[:], in_=bk_d)
            nc.scalar.dma_start(out=bv_sb[:], in_=bv_d)
            nc.scalar.dma_start(out=pad_sb[:], in_=pad_d)
            nc.scalar.dma_start(out=tri_sb[:], in_=tri_d)
            for g in range(4):
                nc.gpsimd.dma_start(out=Wp_sb[:, g * C:(g + 1) * C],
                                    in_=WpT_d[g * 128:(g + 1) * 128, :])
            nc.gpsimd.partition_broadcast(bvb_sb[:], bv_sb[:])

            # ---- work-unit emitters (each: ~1-2us of PE + a DVE drain) ----
            def proj_unit(ch, kind, g):
                """One QKV psum group: 8 accum matmuls + DVE drain."""
                t0 = ch * 512
                xc = xc_t[ch]
                if kind == "q" or kind == "k":
                    W = Wq_sb if kind == "q" else Wk_sb
                    pj = ps1.tile([128, 512], F32, name="pj", tag="p1ps")
                    for ct in range(NCT):
                        nc.tensor.matmul(
                            out=pj[:],
                            lhsT=W[:, ct * 512 + g * 128: ct * 512 + (g + 1) * 128],
                            rhs=xc[:, ct * 512:(ct + 1) * 512],
                            start=(ct == 0), stop=(ct == NCT - 1),
                        )
                    if kind == "q":
                        nc.vector.tensor_scalar(
                            out=QT[:, g * T + t0: g * T + t0 + 512], in0=pj[:],
                            scalar1=SCALE, scalar2=bq_sb[:, g:g + 1],
                            op0=ALU.mult, op1=ALU.add)
                    else:
                        nc.vector.tensor_scalar(
                            out=KT[:, g * T + t0: g * T + t0 + 512], in0=pj[:],
                            scalar1=bk_sb[:, g:g + 1], scalar2=None,
                            op0=ALU.add)
                else:  # v: g is the token sub-tile ts
                    ts = g
                    kt = ch * 4 + ts
                    pj = ps1.tile([128, 512], F32, name="pj", tag="p1ps")
                    for ct in range(NCT):
                        nc.tensor.matmul(
                            out=pj[:],
                            lhsT=xc[:, ct * 512 + ts * 128: ct * 512 + ts * 128 + 128],
                            rhs=Wv_sb[:, ct * 512:(ct + 1) * 512],
                            start=(ct == 0), stop=(ct == NCT - 1),
                        )
                    nc.vector.tensor_tensor(
                        out=Vf[:, kt, :, 0:64],
                        in0=pj.rearrange("p (h c) -> p h c", h=8, c=64),
                        in1=bvb_sb.rearrange("p (h c) -> p h c", h=8, c=64),
                        op=ALU.add)

            def out_unit(tt, oc):
                """One output-projection psum group + drain + DMA."""
                po = ps1.tile([128, 512], F32, name="po", tag="p1ps")
                for g in range(4):
                    nc.tensor.matmul(
                        out=po[:],
                        lhsT=YTg[g][:, tt * 128: tt * 128 + 128],
                        rhs=Wp_sb[:, g * C + oc * 512: g * C + oc * 512 + 512],
                        start=(g == 0), stop=(g == 3),
                    )
                ob = obp.tile([128, 512], F32, name="ob", tag="ob")
                nc.vector.tensor_copy(ob[:], po[:])
                nc.sync.dma_start(
                    out=out_d[tt * 128:(tt + 1) * 128,
                              oc * 512:(oc + 1) * 512],
                    in_=ob[:])

            def proj_chunk_units(ch):
                u = []
                for g in range(4):
                    u.append(("p", ch, "q", g))
                    u.append(("p", ch, "k", g))
                for ts in range(4):
                    u.append(("p", ch, "v", ts))
                return u

            def run_unit(u):
                if u[0] == "p":
                    proj_unit(u[1], u[2], u[3])
                else:
                    out_unit(u[1], u[2])

            # ---- proj(0) up front ----
            for u in proj_chunk_units(0):
                run_unit(u)

            # ---- main loop: attn(qc) with interleaved filler units ----
            for qc in range(4):
                q0 = qc * 512
                kmax = 4 * qc + 4
                fill = []
                if qc < 3:
                    load_x(qc + 1)
                    fill += proj_chunk_units(qc + 1)
                # rounds 1/2 are PE-bound while round 3 is ACT-bound
                # with ~17us of PE slack: park out(1) AND out(2) there
                if qc == 1:
                    fill += [("o", tt, oc)
                             for tt in range(0, 4) for oc in range(2)]
                elif qc == 3:
                    fill += [("o", tt, oc)
                             for tt in range(4, 12) for oc in range(2)]
                nsteps = 4 * (kmax + D)
                # hold filler back until attention has something in flight,
                # then spread evenly over the remaining steps. In the last
                # round, reserve the filler for the back half so it covers
                # the final epilogue's PE bubble.
                lead = min(6, nsteps // 4)
                acc = 0.0
                rate = len(fill) / max(1, nsteps - lead)
                fi = 0

                for g in range(4):
                    gq = g * T
                    oAB = pso.tile([65, 1024], F32, name="oAB", tag="o")
                    e_l = [None] * kmax
                    off_l = [None] * kmax
                    for step in range(kmax + D):
                        gstep = g * (kmax + D) + step
                        if step < kmax:
                            kt = step
                            k0 = kt * 128
                            toff = 128 * (kt - 4 * qc) if kt >= 4 * qc else 0
                            off_l[kt] = toff
                            diag = kt >= 4 * qc
                            sAB = pss.tile([128, 1024], F32, name="sAB",
                                           tag="sAB")
                            nc.tensor.matmul(
                                out=sAB[:, toff:512],
                                lhsT=KT[0:64, gq + k0: gq + k0 + 128],
                                rhs=QT[0:64, gq + q0 + toff: gq + q0 + 512],
                                start=True, stop=True,
                            )
                            nc.tensor.matmul(
                                out=sAB[:, 512 + toff:1024],
                                lhsT=KT[64:128, gq + k0: gq + k0 + 128],
                                rhs=QT[64:128, gq + q0 + toff: gq + q0 + 512],
                                start=True, stop=True, tile_position=(64, 0),
                            )
                            eAB = es.tile([128, 1024], BF16, name="eAB",
                                          tag="eAB")
                            s3 = sAB.rearrange("p (h w) -> p h w", h=2, w=512)
                            e3 = eAB.rearrange("p (h w) -> p h w", h=2, w=512)
                            nc.scalar.activation(
                                e3[:, :, toff:512], s3[:, :, toff:512],
                                AF.Exp, bias=pad_sb[:, kt:kt + 1])
                            if diag:
                                nc.vector.tensor_mul(
                                    eAB[:, toff:toff + 128],
                                    eAB[:, toff:toff + 128], tri_sb[:])
                                nc.vector.tensor_mul(
                                    eAB[:, 512 + toff:512 + toff + 128],
                                    eAB[:, 512 + toff:512 + toff + 128],
                                    tri_sb[:])
                            e_l[kt] = eAB
                        pv = step - D
                        if 0 <= pv < kmax:
                            toff = off_l[pv]
                            vbase = pv * 520
                            nc.tensor.matmul(
                                out=oAB[:, toff:512],
                                lhsT=Vt[:, vbase + 130 * g: vbase + 130 * g + 65],
                                rhs=e_l[pv][:, toff:512],
                                start=(pv == 0), stop=(pv == kmax - 1),
                            )
                            nc.tensor.matmul(
                                out=oAB[:, 512 + toff:1024],
                                lhsT=Vt[:, vbase + 130 * g + 65: vbase + 130 * g + 130],
                                rhs=e_l[pv][:, 512 + toff:1024],
                                start=(pv == 0), stop=(pv == kmax - 1),
                            )
                        # filler between attention steps
                        if gstep >= lead:
                            acc += rate
                            while acc >= 1.0 and fi < len(fill):
                                run_unit(fill[fi])
                                fi += 1
                                acc -= 1.0
                    # epilogue: normalize by rowsum (row 64), write Y^T
                    rs_ = rp.tile([1, 1024], F32, name="rs_", tag="rs_")
                    rr_ = rp.tile([1, 1024], F32, name="rr_", tag="rr_")
                    rbA = rp.tile([64, 512], F32, name="rbA", tag="rbA")
                    rbB = rp.tile([64, 512], F32, name="rbB", tag="rbB")
                    nc.vector.tensor_copy(rs_[:, 0:512], oAB[64:65, 0:512])
                    nc.vector.reciprocal_approx_fast(rr_[:, 0:512], rs_[:, 0:512])
                    nc.vector.tensor_copy(rs_[:, 512:1024], oAB[64:65, 512:1024])
                    nc.gpsimd.partition_broadcast(rbA[:], rr_[:, 0:512])
                    nc.vector.reciprocal_approx_fast(rr_[:, 512:1024], rs_[:, 512:1024])
                    nc.gpsimd.partition_broadcast(rbB[:], rr_[:, 512:1024])
                    nc.vector.tensor_mul(
                        YTg[g][0:64, q0: q0 + 512],
                        oAB[0:64, 0:512], rbA[:])
                    nc.vector.tensor_mul(
                        YTg[g][64:128, q0: q0 + 512],
                        oAB[0:64, 512:1024], rbB[:])
                while fi < len(fill):
                    run_unit(fill[fi])
                    fi += 1

            # ---- tail: last chunk of the output projection ----
            # release the attention pools first so the tail gets a deep
            # psum pool: six units can then hold open accumulation groups
            # and their g0-g2 matmuls cover the final epilogue's latency
            obp.release()
            rp.release()
            es.release()
            pso.release()
            pss.release()
            ptail = tc.alloc_tile_pool(name="ptail", bufs=6, space="PSUM")
            obt = tc.alloc_tile_pool(name="obt", bufs=4)
            for tt in range(12, 16):
                for oc in range(2):
                    po = ptail.tile([128, 512], F32, name="po2", tag="po2")
                    for g in range(4):
                        nc.tensor.matmul(
                            out=po[:],
                            lhsT=YTg[g][:, tt * 128: tt * 128 + 128],
                            rhs=Wp_sb[:, g * C + oc * 512: g * C + oc * 512 + 512],
                            start=(g == 0), stop=(g == 3),
                        )
                    ob = obt.tile([128, 512], F32, name="ob2", tag="ob2")
                    nc.vector.tensor_copy(ob[:], po[:])
                    nc.sync.dma_start(
                        out=out_d[tt * 128:(tt + 1) * 128,
                                  oc * 512:(oc + 1) * 512],
                        in_=ob[:])
            obt.release()
            ptail.release()
            ps1.release()
            xs.release()

    nc.compile()
    return nc


def _in_maps(x, Wk, bk, Wq, bq, Wv, bv, Wp, bp, padding_mask):
    maps = []
    bf16 = ml_dtypes.bfloat16
    rows = np.arange(128)[:, None]
    cols = np.arange(128)[None, :]
    tri01 = (cols >= rows).astype(np.float32)
    for core in range(8):
        b, half = divmod(core, 2)
        hs = slice(half * IC, (half + 1) * IC)
        maps.append({
            "xT": np.ascontiguousarray(x[b].T).astype(bf16),
            "WqT": np.ascontiguousarray(Wq[hs, :].T).astype(bf16),
            "WkT": np.ascontiguousarray(Wk[hs, :].T).astype(bf16),
            "WvT": np.ascontiguousarray(Wv[hs, :].T).astype(bf16),
            "WpT": np.ascontiguousarray(Wp[:, hs].T).astype(bf16),
            "bqs": np.ascontiguousarray((bq[hs] * SCALE).reshape(4, 128).T),
            "bks": np.ascontiguousarray(bk[hs].reshape(4, 128).T),
            "bvr": bv[hs].reshape(1, IC).copy(),
            "padb": np.ascontiguousarray(
                np.where(padding_mask[b] != 0, 0.0, NEG)
                .astype(np.float32).reshape(NKT, 128).T),
            "tri01": tri01.astype(bf16),
            "ones8": np.ones((128, 8), bf16),
        })
    return maps


def _run(inputs, trace=False, **kw):
    if "nc" not in _CACHE:
        _CACHE["nc"] = _build()
    nc = _CACHE["nc"]
    ins = {k: np.asarray(v, dtype=np.float32) if k != "padding_mask"
           else np.asarray(v) for k, v in inputs.items()}
    maps = _in_maps(**ins)
    res = run_bass_kernel_spmd(nc, maps, core_ids=list(range(8)), trace=trace, **kw)
    bp = np.asarray(inputs["bp"], np.float32)
    y = np.empty((B, T, C), np.float32)
    for b in range(B):
        y[b] = res.results[2 * b]["out"] + res.results[2 * b + 1]["out"] + bp
    return y, res


def kernel(**inputs):
    y, _ = _run(inputs, trace=False)
    return y
